# revision 12
# baseline (speedup 1.0000x reference)
"""Trainium2 Bass kernel v2 for nn_Discriminator.

Structure (8 NeuronCores, SPMD, dst-node sharded graph + row-sharded Wi):
  - Linear GCN branches fold to 4 SpMV applications of A_hat per branch;
    (A+I) slices held SBUF-resident as fp8 [128, DCH*SCH*128] blocks in
    dst-major order so iteration-1 matmuls pipeline behind the M DMA.
  - X is sharded: each core computes u0 for its 1024 nodes, one combined
    AllGather replicates a0 = dis*u0 for both branches.
  - fc_inter folded with fc_final: y = hcat @ (Wi Wf_top) + bi Wf_top
    + meta Wf_bot + bf.  Wi Wf_top is computed on-device by streaming
    host-transposed bf16 Wi^T bands [128, 2048] and accumulating
    [2048, 17] in PSUM -- independent of the graph output, so band
    matmuls interleave between graph iterations and hide AllGather
    latency while the DMA engine streams the 32MB of Wi continuously.
  - Final: 16 tiny matmuls contract hcat with WiWf, +bias terms, then a
    17-float AllReduce.
"""
import numpy as np
import ml_dtypes

N = 8192
F = 128
E = 262144
META = 64
LAM = 16
NC = 8
SLICE = N // NC          # 1024 dst nodes per core
DCH = SLICE // 128       # 8 dst chunks per core
SCH = N // 128           # 64 src chunks
OUTD = LAM + 1           # 17
RLOC = 2 * SLICE         # 2048 local wi rows
JCH = N // 128           # 64 j chunks of wi columns

_compiled = None


def _build_bass(nrep=1, hwrep=1, skip_wi=False, skip_graph=False, no_collective=False,
                band_sched=None):
    import concourse.bass as bass
    import concourse.mybir as mybir
    import concourse.tile as tile
    from concourse import bacc

    dt = mybir.dt
    nc = bacc.Bacc(None, target_bir_lowering=False, debug=False)

    def din(name, shape, dtype=dt.float32):
        return nc.declare_dram_parameter(name, list(shape), dtype, isOutput=False)

    mt1 = din("mt1", [128, DCH * SCH * 128], dt.float8e4)
    mt2 = din("mt2", [128, DCH * SCH * 128], dt.float8e4)
    xs1_in = din("xs1", [F, SLICE])
    xs2_in = din("xs2", [F, SLICE])
    deg1l_in = din("deg1l", [128, DCH])
    deg2l_in = din("deg2l", [128, DCH])
    wit_in = din("wit", [N, RLOC], dt.bfloat16)
    wfc_in = din("wfc", [128, JCH * OUTD])
    wfb_in = din("wfb", [META, OUTD])
    meta_in = din("metac", [META, 1])
    bic_in = din("bic", [128, JCH])
    bfc_in = din("bfc", [OUTD, 1])
    w1t_in = din("w1t", [8, 128])
    w2t_in = din("w2t", [4, 8])
    w3t_in = din("w3t", [2, 4])
    w4_in = din("w4", [2, 1])
    b1_in = din("b1", [8, 1])
    b2_in = din("b2", [4, 1])
    b3_in = din("b3", [2, 1])
    b4_in = din("b4", [1, 1])
    y_out = nc.declare_dram_parameter("y", [OUTD, 1], dt.float32, isOutput=True)

    with tile.TileContext(nc) as tc:
        with (
            tc.tile_pool(name="small", bufs=1) as sp,
            tc.tile_pool(name="dram", bufs=1, space="DRAM") as dram,
            tc.tile_pool(name="mtp", bufs=1) as mtp,
        ):
            ones_row = sp.tile([1, 128], dt.float32)
            nc.gpsimd.memset(ones_row[:], 1.0)

            # ---- tiny weight chain: w = W1 W2 W3 W4, c_k bias scalars ----
            w1t = sp.tile([8, 128], dt.float32)
            w2t = sp.tile([4, 8], dt.float32)
            w3t = sp.tile([2, 4], dt.float32)
            w4 = sp.tile([2, 1], dt.float32)
            b1 = sp.tile([8, 1], dt.float32)
            b2 = sp.tile([4, 1], dt.float32)
            b3 = sp.tile([2, 1], dt.float32)
            b4 = sp.tile([1, 1], dt.float32)
            for t, i in ((w1t, w1t_in), (w2t, w2t_in), (w3t, w3t_in), (w4, w4_in),
                         (b1, b1_in), (b2, b2_in), (b3, b3_in), (b4, b4_in)):
                nc.sync.dma_start(out=t[:], in_=i[:])

            with tc.tile_pool(name="ps_setup", bufs=2,
                              space=bass.MemorySpace.PSUM) as psp:
                def ps_tile():
                    ps_m = psp.tile([128, 64], dt.float32)
                    return ps_m

                ps_m = ps_tile()
                nc.tensor.matmul(ps_m[0:4, 0:1], w3t[:], w4[:])
                v2 = sp.tile([4, 1], dt.float32)
                nc.vector.tensor_copy(v2[:], ps_m[0:4, 0:1])

                ps_m2 = ps_tile()
                nc.tensor.matmul(ps_m2[0:8, 0:1], w2t[:], v2[:])
                v1 = sp.tile([8, 1], dt.float32)
                nc.vector.tensor_copy(v1[:], ps_m2[0:8, 0:1])

                ps_m3 = ps_tile()
                nc.tensor.matmul(ps_m3[:, 0:1], w1t[:], v1[:])
                wcol_bf = sp.tile([128, 1], dt.bfloat16)
                nc.vector.tensor_copy(wcol_bf[:], ps_m3[:, 0:1])

                c_sb = sp.tile([1, 4], dt.float32)
                ps_m4 = ps_tile()
                nc.tensor.matmul(ps_m4[0:1, 0:1], b1[:], v1[:])
                nc.vector.tensor_copy(c_sb[0:1, 0:1], ps_m4[0:1, 0:1])
                ps_m5 = ps_tile()
                nc.tensor.matmul(ps_m5[0:1, 0:1], b2[:], v2[:])
                nc.vector.tensor_copy(c_sb[0:1, 1:2], ps_m5[0:1, 0:1])
                ps_m6 = ps_tile()
                nc.tensor.matmul(ps_m6[0:1, 0:1], b3[:], w4[:])
                nc.vector.tensor_copy(c_sb[0:1, 2:3], ps_m6[0:1, 0:1])
                nc.vector.tensor_copy(c_sb[0:1, 3:4], b4[:])

                ps_m7 = ps_tile()
                nc.tensor.matmul(ps_m7[:, 0:4], ones_row[:], c_sb[:])
                c_cols = sp.tile([128, 4], dt.float32)
                nc.vector.tensor_copy(c_cols[:], ps_m7[:, 0:4])

            # ---- shared small tensors ----
            wfc_sb = sp.tile([128, JCH * OUTD], dt.float32)
            nc.sync.dma_start(out=wfc_sb[:], in_=wfc_in[:])
            wfc_bf = sp.tile([128, JCH * OUTD], dt.bfloat16)
            nc.vector.tensor_copy(wfc_bf[:], wfc_sb[:])
            wfb_sb = sp.tile([META, OUTD], dt.float32)
            nc.sync.dma_start(out=wfb_sb[:], in_=wfb_in[:])
            metac = sp.tile([META, 1], dt.float32)
            nc.sync.dma_start(out=metac[:], in_=meta_in[:])
            meta_s = sp.tile([META, 1], dt.float32)
            nc.vector.tensor_scalar_mul(meta_s[:], metac[:], 1.0 / NC)
            bic_sb = sp.tile([128, JCH], dt.float32)
            nc.sync.dma_start(out=bic_sb[:], in_=bic_in[:])
            bic_s = sp.tile([128, JCH], dt.float32)
            nc.vector.tensor_scalar_mul(bic_s[:], bic_sb[:], 1.0 / NC)
            bfc_sb = sp.tile([OUTD, 1], dt.float32)
            nc.sync.dma_start(out=bfc_sb[:], in_=bfc_in[:])
            bfc_s = sp.tile([OUTD, 1], dt.float32)
            nc.vector.tensor_scalar_mul(bfc_s[:], bfc_sb[:], 1.0 / NC)

            mts = {1: mtp.tile([128, DCH * SCH * 128], dt.float8e4, name="mts1"),
                   2: mtp.tile([128, DCH * SCH * 128], dt.float8e4, name="mts2")}

            import contextlib

            def emit_rep(rep, rp, bandp, psg, pswf, psf):
                if True:
                    # ---- DMA: X slices, then M chunks (dst-major) ----
                    xs_bf = {}
                    for b, xin in ((1, xs1_in), (2, xs2_in)):
                        xf = rp.tile([F, SLICE], dt.float32, name=f"xf{b}")
                        nc.sync.dma_start(out=xf[:], in_=xin[:])
                        xbf = rp.tile([F, SLICE], dt.bfloat16, name=f"xbf{b}")
                        nc.vector.tensor_copy(xbf[:], xf[:])
                        xs_bf[b] = xbf
                    for b, src in ((1, mt1), (2, mt2)):
                        for d in range(DCH):
                            lo, hi = d * SCH * 128, (d + 1) * SCH * 128
                            nc.sync.dma_start(out=mts[b][:, lo:hi], in_=src[:, lo:hi])

                    # ---- dis = 1/sqrt(deg), local slice only ----
                    dis = {}
                    for b, dl in ((1, deg1l_in), (2, deg2l_in)):
                        degl = rp.tile([128, DCH], dt.float32, name=f"degl{b}")
                        nc.sync.dma_start(out=degl[:], in_=dl[:])
                        disl = rp.tile([128, DCH], dt.float32, name=f"disl{b}")
                        nc.vector.reciprocal(disl[:], degl[:])
                        nc.scalar.activation(disl[:], disl[:],
                                             mybir.ActivationFunctionType.Sqrt)
                        dis[b] = disl

                    # ---- u0 slice = X_k w ; a0_l = disl * u0 ; combined AG ----
                    a_bf = {1: rp.tile([128, SCH], dt.bfloat16, name="a_bf1"),
                            2: rp.tile([128, SCH], dt.bfloat16, name="a_bf2")}
                    a_f = {1: rp.tile([128, SCH], dt.float32, name="a_f1"),
                           2: rp.tile([128, SCH], dt.float32, name="a_f2")}
                    ps_u = psg.tile([128, 2 * DCH], dt.float32)
                    for b in (1, 2):
                        for c in range(DCH):
                            nc.tensor.matmul(
                                ps_u[:, (b - 1) * DCH + c:(b - 1) * DCH + c + 1],
                                xs_bf[b][:, c * 128:(c + 1) * 128], wcol_bf[:])
                    ag0_l = rp.tile([128, 2 * DCH], dt.float32, name="ag0_l")
                    for b in (1, 2):
                        sl = slice((b - 1) * DCH, b * DCH)
                        nc.vector.tensor_tensor(ag0_l[:, sl], ps_u[:, sl],
                                                dis[b][:], mybir.AluOpType.mult)
                    ag0_i = dram.tile([128, 2 * DCH], dt.float32, name=f"ag0_i_{rep}")
                    ag0_o = dram.tile([128 * NC, 2 * DCH], dt.float32,
                                      name=f"ag0_o_{rep}")
                    nc.sync.dma_start(out=ag0_i[:], in_=ag0_l[:])
                    if no_collective:
                        nc.sync.dma_start(out=ag0_o[0:128, :], in_=ag0_i[:])
                    else:
                        nc.gpsimd.collective_compute(
                            "AllGather", mybir.AluOpType.bypass,
                            replica_groups=[list(range(NC))],
                            ins=[ag0_i[:].opt()], outs=[ag0_o[:].opt()])
                    for b in (1, 2):
                        sl = slice((b - 1) * DCH, b * DCH)
                        nc.sync.dma_start(
                            out=a_f[b][:].rearrange("p (r j) -> p r j", r=NC),
                            in_=ag0_o[:, sl].rearrange("(r p) j -> p r j", r=NC))
                        nc.vector.tensor_copy(a_bf[b][:], a_f[b][:])

                    # ---- WiWf band machinery ----
                    ps_wf = pswf.tile([128, 16 * OUTD], dt.float32)
                    band_state = [0]

                    def emit_bands(cnt):
                        if skip_wi:
                            return
                        for _ in range(cnt):
                            jc = band_state[0]
                            if jc >= JCH:
                                return
                            band_state[0] += 1
                            band = bandp.tile([128, RLOC], dt.bfloat16)
                            nc.sync.dma_start(out=band[:],
                                              in_=wit_in[jc * 128:(jc + 1) * 128, :])
                            for rc in range(16):
                                nc.tensor.matmul(
                                    ps_wf[:, rc * OUTD:(rc + 1) * OUTD],
                                    band[:, rc * 128:(rc + 1) * 128],
                                    wfc_bf[:, jc * OUTD:(jc + 1) * OUTD],
                                    start=(jc == 0), stop=(jc == JCH - 1))

                    hcat_bf = rp.tile([128, 16], dt.bfloat16, name="hcat")
                    if skip_graph:
                        nc.gpsimd.memset(hcat_bf[:], 0.001)

                    # ---- graph iterations, WiWf bands interleaved ----
                    sched = band_sched or [7] * 8
                    slot = 0
                    for k in range(1, 5):
                        if skip_graph:
                            break
                        for b in (1, 2):
                            disl = dis[b]
                            ps_t = psg.tile([128, DCH], dt.float32, name="ps_t")
                            mtb = mts[b]
                            for d in range(DCH):
                                for s in range(SCH):
                                    off = (d * SCH + s) * 128
                                    nc.tensor.matmul(
                                        ps_t[:, d:d + 1], mtb[:, off:off + 128],
                                        a_bf[b][:, s:s + 1],
                                        start=(s == 0), stop=(s == SCH - 1))
                            t_cols = rp.tile([128, DCH], dt.float32,
                                             name=f"t_cols_{b}_{k}")
                            nc.vector.tensor_tensor(t_cols[:], ps_t[:], disl[:],
                                                    mybir.AluOpType.mult)
                            nc.vector.tensor_scalar_add(t_cols[:], t_cols[:],
                                                        c_cols[:, k - 1:k])
                            if k < 4:
                                a_l = rp.tile([128, DCH], dt.float32,
                                              name=f"a_l_{b}_{k}")
                                nc.vector.tensor_tensor(a_l[:], t_cols[:], disl[:],
                                                        mybir.AluOpType.mult)
                                ag_i = dram.tile([128, DCH], dt.float32,
                                                 name=f"ag_i_{b}_{k}_{rep}")
                                ag_o = dram.tile([128 * NC, DCH], dt.float32,
                                                 name=f"ag_o_{b}_{k}_{rep}")
                                nc.sync.dma_start(out=ag_i[:], in_=a_l[:])
                                if no_collective:
                                    nc.sync.dma_start(out=ag_o[0:128, :], in_=ag_i[:])
                                else:
                                    nc.gpsimd.collective_compute(
                                        "AllGather", mybir.AluOpType.bypass,
                                        replica_groups=[list(range(NC))],
                                        ins=[ag_i[:].opt()], outs=[ag_o[:].opt()])
                                nc.sync.dma_start(
                                    out=a_f[b][:].rearrange("p (r j) -> p r j", r=NC),
                                    in_=ag_o[:].rearrange("(r p) j -> p r j", r=NC))
                                nc.vector.tensor_copy(a_bf[b][:], a_f[b][:])
                            else:
                                nc.vector.tensor_copy(
                                    hcat_bf[:, (b - 1) * DCH:b * DCH], t_cols[:])
                            emit_bands(sched[slot] if slot < len(sched) else 7)
                            slot += 1
                    emit_bands(JCH)  # whatever remains

                    # ---- bias-path fold: (bi/NC) Wf_top + (meta/NC) Wf_bot ----
                    ps_bi = psf.tile([OUTD, 1], dt.float32)
                    for jc in range(JCH):
                        nc.tensor.matmul(ps_bi[:], wfc_sb[:, jc * OUTD:(jc + 1) * OUTD],
                                         bic_s[:, jc:jc + 1],
                                         start=(jc == 0), stop=False)
                    nc.tensor.matmul(ps_bi[:], wfb_sb[:], meta_s[:],
                                     start=False, stop=True)

                    # ---- hcat contraction with WiWf ----
                    o_part = rp.tile([OUTD, 1], dt.float32, name="o_part")
                    if skip_wi:
                        nc.vector.tensor_tensor(o_part[:], bfc_s[:], ps_bi[:],
                                                mybir.AluOpType.add)
                    else:
                        wiwf_bf = rp.tile([128, 16 * OUTD], dt.bfloat16, name="wiwf")
                        nc.vector.tensor_copy(wiwf_bf[:], ps_wf[:])
                        ps_y = psf.tile([OUTD, 1], dt.float32)
                        for rc in range(16):
                            nc.tensor.matmul(ps_y[:],
                                             wiwf_bf[:, rc * OUTD:(rc + 1) * OUTD],
                                             hcat_bf[:, rc:rc + 1],
                                             start=(rc == 0), stop=(rc == 15))
                        nc.vector.tensor_tensor(o_part[:], bfc_s[:], ps_bi[:],
                                                mybir.AluOpType.add)
                        nc.vector.tensor_tensor(o_part[:], o_part[:], ps_y[:],
                                                mybir.AluOpType.add)

                    ar_i = dram.tile([OUTD, 1], dt.float32, name=f"ar_i_{rep}")
                    ar_o = dram.tile([OUTD, 1], dt.float32, name=f"ar_o_{rep}")
                    nc.sync.dma_start(out=ar_i[:], in_=o_part[:])
                    if no_collective:
                        nc.sync.dma_start(out=ar_o[:], in_=ar_i[:])
                    else:
                        nc.gpsimd.collective_compute(
                            "AllReduce", mybir.AluOpType.add,
                            replica_groups=[list(range(NC))],
                            ins=[ar_i[:].opt()], outs=[ar_o[:].opt()])
                    nc.sync.dma_start(out=y_out[:], in_=ar_o[:])

            if hwrep > 1:
                with (
                    tc.tile_pool(name="rp0", bufs=1) as rp,
                    tc.tile_pool(name="bandp0", bufs=8) as bandp,
                    tc.tile_pool(name="psg0", bufs=2,
                                 space=bass.MemorySpace.PSUM) as psg,
                    tc.tile_pool(name="pswf0", bufs=1,
                                 space=bass.MemorySpace.PSUM) as pswf,
                    tc.tile_pool(name="psf0", bufs=1,
                                 space=bass.MemorySpace.PSUM) as psf,
                ):
                    with tc.For_i(0, hwrep):
                        emit_rep(0, rp, bandp, psg, pswf, psf)
            else:
                for rep in range(nrep):
                    with (
                        tc.tile_pool(name=f"rp{rep}", bufs=1) as rp,
                        tc.tile_pool(name=f"bandp{rep}", bufs=8) as bandp,
                        tc.tile_pool(name=f"psg{rep}", bufs=2,
                                     space=bass.MemorySpace.PSUM) as psg,
                        tc.tile_pool(name=f"pswf{rep}", bufs=1,
                                     space=bass.MemorySpace.PSUM) as pswf,
                        tc.tile_pool(name=f"psf{rep}", bufs=1,
                                     space=bass.MemorySpace.PSUM) as psf,
                    ):
                        emit_rep(rep, rp, bandp, psg, pswf, psf)

    nc.compile()
    return nc


def _host_prep(x1, x2, meta, W1, b1, W2, b2, W3, b3, W4, b4, Wi, bi, Wf, bf,
               edge_index1, edge_index2):
    """Per-core input maps (sharding + layout/dtype only)."""
    f32 = np.float32
    bf16 = ml_dtypes.bfloat16

    def graph_side(edge_index):
        src = np.asarray(edge_index[0], np.int64)
        dst = np.asarray(edge_index[1], np.int64)
        M = np.zeros((N, N), np.int16)        # [dst, src] counts
        np.add.at(M, (dst, src), 1)
        M[np.arange(N), np.arange(N)] += 1    # self loops
        deg = (np.bincount(dst, minlength=N) + 1).astype(f32)
        mts, degls = [], []
        for k in range(NC):
            sl = M[k * SLICE:(k + 1) * SLICE, :]          # [1024 dst, 8192 src]
            MT = np.ascontiguousarray(sl.T)               # [8192 src, 1024 dst]
            til = MT.reshape(SCH, 128, DCH, 128)          # (s, q, d, p)
            arr = til.transpose(1, 2, 0, 3)               # (q, d, s, p) dst-major
            mts.append(np.ascontiguousarray(arr.reshape(128, DCH * SCH * 128))
                       .astype(ml_dtypes.float8_e4m3))
            dl = deg[k * SLICE:(k + 1) * SLICE].reshape(DCH, 128).T
            degls.append(np.ascontiguousarray(dl))
        return mts, degls

    mts1, deg1l = graph_side(edge_index1)
    mts2, deg2l = graph_side(edge_index2)

    xt1 = np.asarray(x1, f32).T                           # [F, N]
    xt2 = np.asarray(x2, f32).T

    Wi = np.asarray(Wi, f32)
    WiT = np.ascontiguousarray(Wi.T).astype(bf16)         # [N, 2N]
    Wf = np.asarray(Wf, f32)
    wf_top = Wf[:N]
    wfc = np.ascontiguousarray(
        wf_top.reshape(JCH, 128, OUTD).transpose(1, 0, 2).reshape(128, JCH * OUTD))
    wfb = np.ascontiguousarray(Wf[N:])
    bic = np.ascontiguousarray(np.asarray(bi, f32).reshape(JCH, 128).T)

    common = {
        "wfc": wfc, "wfb": wfb,
        "metac": np.asarray(meta, f32).reshape(META, 1),
        "bic": bic,
        "bfc": np.asarray(bf, f32).reshape(OUTD, 1),
        "w1t": np.ascontiguousarray(np.asarray(W1, f32).T),
        "w2t": np.ascontiguousarray(np.asarray(W2, f32).T),
        "w3t": np.ascontiguousarray(np.asarray(W3, f32).T),
        "w4": np.asarray(W4, f32).reshape(2, 1),
        "b1": np.asarray(b1, f32).reshape(8, 1),
        "b2": np.asarray(b2, f32).reshape(4, 1),
        "b3": np.asarray(b3, f32).reshape(2, 1),
        "b4": np.asarray(b4, f32).reshape(1, 1),
    }
    in_maps = []
    for k in range(NC):
        m = dict(common)
        m["mt1"] = mts1[k]
        m["mt2"] = mts2[k]
        m["deg1l"] = deg1l[k]
        m["deg2l"] = deg2l[k]
        m["xs1"] = np.ascontiguousarray(xt1[:, k * SLICE:(k + 1) * SLICE])
        m["xs2"] = np.ascontiguousarray(xt2[:, k * SLICE:(k + 1) * SLICE])
        m["wit"] = np.ascontiguousarray(
            np.concatenate([WiT[:, k * SLICE:(k + 1) * SLICE],
                            WiT[:, N + k * SLICE:N + (k + 1) * SLICE]], axis=1))
        in_maps.append(m)
    return in_maps


def kernel(**inputs) -> np.ndarray:
    global _compiled
    in_maps = _host_prep(**inputs)
    if _compiled is None:
        _compiled = _build_bass()
    from concourse.bass_utils import run_bass_kernel_spmd
    res = run_bass_kernel_spmd(_compiled, in_maps, core_ids=list(range(NC)))
    return res.results[0]["y"].reshape(OUTD).astype(np.float32)



# revision 15
# speedup vs baseline: 1.1724x; 1.1724x over previous
"""Trainium2 Bass kernel v3 (B-operator) for nn_Discriminator.

Structure (8 NeuronCores, SPMD, dst-node sharded graph + row-sharded Wi):
  - Linear GCN branches fold algebraically: with A_hat = Ds(A+I)Ds and
    B = (A+I) D (A+I) (the 2-hop operator, built host-side from the edge
    list with scipy.sparse -- graph structure only, like the degree
    computation), each branch needs only THREE SpMV passes over B per
    graph instead of four over (A+I):
      z  = B(c1 D q + c2 dis)   [input-independent, covers AG0 latency]
      t1 = B a0;  t2 = B(D t1)
      h  = Ds t2 + Ds z + c3 Ds q + c4,   q = (A+I) dis (host, structural)
    B slices are SBUF-resident fp8e4m3 [128, DCH*SCH*128] dst-major
    blocks (same bytes/layout as A+I; its quantization noise is
    negligible end-to-end).  Collectives drop from 8 to 4: AG0 hides
    under the z-passes, AG(t1_b) under the other branch's block.
  - fc_inter folded with fc_final: y = hcat @ (Wi Wf_top) + bi Wf_top
    + meta Wf_bot + bf.  Wi Wf_top is computed on-device by streaming
    host-transposed bf16 Wi^T bands [128, 2048] and accumulating
    [2048, 17] in PSUM -- independent of the graph output, so band
    matmuls interleave between graph passes and hide AllGather latency
    while the DMA engine streams the 32MB of Wi continuously.
  - Final: 16 tiny matmuls contract hcat with WiWf, +bias terms, then a
    17-float AllReduce.
"""
import numpy as np
import ml_dtypes

N = 8192
F = 128
E = 262144
META = 64
LAM = 16
NC = 8
SLICE = N // NC          # 1024 dst nodes per core
DCH = SLICE // 128       # 8 dst chunks per core
SCH = N // 128           # 64 src chunks
OUTD = LAM + 1           # 17
RLOC = 2 * SLICE         # 2048 local wi rows
JCH = N // 128           # 64 j chunks of wi columns

_compiled = None


def _build_bass(nrep=1, hwrep=1, skip_wi=False, skip_graph=False, no_collective=False,
                band_sched=None):
    import concourse.bass as bass
    import concourse.mybir as mybir
    import concourse.tile as tile
    from concourse import bacc

    dt = mybir.dt
    nc = bacc.Bacc(None, target_bir_lowering=False, debug=False)

    def din(name, shape, dtype=dt.float32):
        return nc.declare_dram_parameter(name, list(shape), dtype, isOutput=False)

    mt1 = din("mt1", [128, DCH * SCH * 128], dt.float8e4)
    mt2 = din("mt2", [128, DCH * SCH * 128], dt.float8e4)
    xs1_in = din("xs1", [F, SLICE])
    xs2_in = din("xs2", [F, SLICE])
    deg1l_in = din("deg1l", [128, DCH])
    deg2l_in = din("deg2l", [128, DCH])
    qdf1_in = din("qdf1", [128, SCH])
    qdf2_in = din("qdf2", [128, SCH])
    disf1_in = din("disf1", [128, SCH])
    disf2_in = din("disf2", [128, SCH])
    qdl1_in = din("qdl1", [128, DCH])
    qdl2_in = din("qdl2", [128, DCH])
    wit_in = din("wit", [N, RLOC], dt.bfloat16)
    wfc_in = din("wfc", [128, JCH * OUTD])
    wfb_in = din("wfb", [META, OUTD])
    meta_in = din("metac", [META, 1])
    bic_in = din("bic", [128, JCH])
    bfc_in = din("bfc", [OUTD, 1])
    w1t_in = din("w1t", [8, 128])
    w2t_in = din("w2t", [4, 8])
    w3t_in = din("w3t", [2, 4])
    w4_in = din("w4", [2, 1])
    b1_in = din("b1", [8, 1])
    b2_in = din("b2", [4, 1])
    b3_in = din("b3", [2, 1])
    b4_in = din("b4", [1, 1])
    y_out = nc.declare_dram_parameter("y", [OUTD, 1], dt.float32, isOutput=True)

    with tile.TileContext(nc) as tc:
        with (
            tc.tile_pool(name="small", bufs=1) as sp,
            tc.tile_pool(name="dram", bufs=1, space="DRAM") as dram,
            tc.tile_pool(name="mtp", bufs=1) as mtp,
        ):
            ones_row = sp.tile([1, 128], dt.float32)
            nc.gpsimd.memset(ones_row[:], 1.0)

            # ---- tiny weight chain: w = W1 W2 W3 W4, c_k bias scalars ----
            w1t = sp.tile([8, 128], dt.float32)
            w2t = sp.tile([4, 8], dt.float32)
            w3t = sp.tile([2, 4], dt.float32)
            w4 = sp.tile([2, 1], dt.float32)
            b1 = sp.tile([8, 1], dt.float32)
            b2 = sp.tile([4, 1], dt.float32)
            b3 = sp.tile([2, 1], dt.float32)
            b4 = sp.tile([1, 1], dt.float32)
            for t, i in ((w1t, w1t_in), (w2t, w2t_in), (w3t, w3t_in), (w4, w4_in),
                         (b1, b1_in), (b2, b2_in), (b3, b3_in), (b4, b4_in)):
                nc.sync.dma_start(out=t[:], in_=i[:])

            with tc.tile_pool(name="ps_setup", bufs=2,
                              space=bass.MemorySpace.PSUM) as psp:
                def ps_tile():
                    ps_m = psp.tile([128, 64], dt.float32)
                    return ps_m

                ps_m = ps_tile()
                nc.tensor.matmul(ps_m[0:4, 0:1], w3t[:], w4[:])
                v2 = sp.tile([4, 1], dt.float32)
                nc.vector.tensor_copy(v2[:], ps_m[0:4, 0:1])

                ps_m2 = ps_tile()
                nc.tensor.matmul(ps_m2[0:8, 0:1], w2t[:], v2[:])
                v1 = sp.tile([8, 1], dt.float32)
                nc.vector.tensor_copy(v1[:], ps_m2[0:8, 0:1])

                ps_m3 = ps_tile()
                nc.tensor.matmul(ps_m3[:, 0:1], w1t[:], v1[:])
                wcol_bf = sp.tile([128, 1], dt.bfloat16)
                nc.vector.tensor_copy(wcol_bf[:], ps_m3[:, 0:1])

                c_sb = sp.tile([1, 4], dt.float32)
                ps_m4 = ps_tile()
                nc.tensor.matmul(ps_m4[0:1, 0:1], b1[:], v1[:])
                nc.vector.tensor_copy(c_sb[0:1, 0:1], ps_m4[0:1, 0:1])
                ps_m5 = ps_tile()
                nc.tensor.matmul(ps_m5[0:1, 0:1], b2[:], v2[:])
                nc.vector.tensor_copy(c_sb[0:1, 1:2], ps_m5[0:1, 0:1])
                ps_m6 = ps_tile()
                nc.tensor.matmul(ps_m6[0:1, 0:1], b3[:], w4[:])
                nc.vector.tensor_copy(c_sb[0:1, 2:3], ps_m6[0:1, 0:1])
                nc.vector.tensor_copy(c_sb[0:1, 3:4], b4[:])

                ps_m7 = ps_tile()
                nc.tensor.matmul(ps_m7[:, 0:4], ones_row[:], c_sb[:])
                c_cols = sp.tile([128, 4], dt.float32)
                nc.vector.tensor_copy(c_cols[:], ps_m7[:, 0:4])

            # ---- shared small tensors ----
            wfc_sb = sp.tile([128, JCH * OUTD], dt.float32)
            nc.sync.dma_start(out=wfc_sb[:], in_=wfc_in[:])
            wfc_bf = sp.tile([128, JCH * OUTD], dt.bfloat16)
            nc.vector.tensor_copy(wfc_bf[:], wfc_sb[:])
            wfb_sb = sp.tile([META, OUTD], dt.float32)
            nc.sync.dma_start(out=wfb_sb[:], in_=wfb_in[:])
            metac = sp.tile([META, 1], dt.float32)
            nc.sync.dma_start(out=metac[:], in_=meta_in[:])
            meta_s = sp.tile([META, 1], dt.float32)
            nc.vector.tensor_scalar_mul(meta_s[:], metac[:], 1.0 / NC)
            bic_sb = sp.tile([128, JCH], dt.float32)
            nc.sync.dma_start(out=bic_sb[:], in_=bic_in[:])
            bic_s = sp.tile([128, JCH], dt.float32)
            nc.vector.tensor_scalar_mul(bic_s[:], bic_sb[:], 1.0 / NC)
            bfc_sb = sp.tile([OUTD, 1], dt.float32)
            nc.sync.dma_start(out=bfc_sb[:], in_=bfc_in[:])
            bfc_s = sp.tile([OUTD, 1], dt.float32)
            nc.vector.tensor_scalar_mul(bfc_s[:], bfc_sb[:], 1.0 / NC)

            mts = {1: mtp.tile([128, DCH * SCH * 128], dt.float8e4, name="mts1"),
                   2: mtp.tile([128, DCH * SCH * 128], dt.float8e4, name="mts2")}

            import contextlib

            def emit_rep(rep, rp, bandp, psg, pswf, psf):
                if True:
                    # ---- DMA: X slices, then M chunks (dst-major) ----
                    xs_bf = {}
                    for b, xin in ((1, xs1_in), (2, xs2_in)):
                        xf = rp.tile([F, SLICE], dt.float32, name=f"xf{b}")
                        nc.sync.dma_start(out=xf[:], in_=xin[:])
                        xbf = rp.tile([F, SLICE], dt.bfloat16, name=f"xbf{b}")
                        nc.vector.tensor_copy(xbf[:], xf[:])
                        xs_bf[b] = xbf
                    for b, src in ((1, mt1), (2, mt2)):
                        for d in range(DCH):
                            lo, hi = d * SCH * 128, (d + 1) * SCH * 128
                            nc.sync.dma_start(out=mts[b][:, lo:hi], in_=src[:, lo:hi])

                    # ---- dis = 1/sqrt(deg), local slice; structural vectors ----
                    dis, dl2s, qdf, disf, qdl = {}, {}, {}, {}, {}
                    for b, dl, qf, df, ql in ((1, deg1l_in, qdf1_in, disf1_in, qdl1_in),
                                              (2, deg2l_in, qdf2_in, disf2_in, qdl2_in)):
                        degl = rp.tile([128, DCH], dt.float32, name=f"degl{b}")
                        nc.sync.dma_start(out=degl[:], in_=dl[:])
                        disl = rp.tile([128, DCH], dt.float32, name=f"disl{b}")
                        nc.vector.reciprocal(disl[:], degl[:])
                        nc.scalar.activation(disl[:], disl[:],
                                             mybir.ActivationFunctionType.Sqrt)
                        dis[b] = disl
                        dl2 = rp.tile([128, DCH], dt.float32, name=f"dl2_{b}")
                        nc.vector.tensor_tensor(dl2[:], disl[:], disl[:],
                                                mybir.AluOpType.mult)
                        dl2s[b] = dl2
                        t_qf = rp.tile([128, SCH], dt.float32, name=f"qdf{b}")
                        nc.sync.dma_start(out=t_qf[:], in_=qf[:])
                        qdf[b] = t_qf
                        t_df = rp.tile([128, SCH], dt.float32, name=f"disf{b}")
                        nc.sync.dma_start(out=t_df[:], in_=df[:])
                        disf[b] = t_df
                        t_ql = rp.tile([128, DCH], dt.float32, name=f"qdl{b}")
                        nc.sync.dma_start(out=t_ql[:], in_=ql[:])
                        qdl[b] = t_ql

                    # ---- u0 slice = X_k w ; a0_l = disl * u0 ; combined AG ----
                    a_bf = {1: rp.tile([128, SCH], dt.bfloat16, name="a_bf1"),
                            2: rp.tile([128, SCH], dt.bfloat16, name="a_bf2")}
                    a_f = {1: rp.tile([128, SCH], dt.float32, name="a_f1"),
                           2: rp.tile([128, SCH], dt.float32, name="a_f2")}
                    ps_u = psg.tile([128, 2 * DCH], dt.float32)
                    for b in (1, 2):
                        for c in range(DCH):
                            nc.tensor.matmul(
                                ps_u[:, (b - 1) * DCH + c:(b - 1) * DCH + c + 1],
                                xs_bf[b][:, c * 128:(c + 1) * 128], wcol_bf[:])
                    ag0_l = rp.tile([128, 2 * DCH], dt.float32, name="ag0_l")
                    for b in (1, 2):
                        sl = slice((b - 1) * DCH, b * DCH)
                        nc.vector.tensor_tensor(ag0_l[:, sl], ps_u[:, sl],
                                                dis[b][:], mybir.AluOpType.mult)
                    ag0_i = dram.tile([128, 2 * DCH], dt.float32, name=f"ag0_i_{rep}")
                    ag0_o = dram.tile([128 * NC, 2 * DCH], dt.float32,
                                      name=f"ag0_o_{rep}")
                    nc.sync.dma_start(out=ag0_i[:], in_=ag0_l[:])
                    if no_collective:
                        nc.sync.dma_start(out=ag0_o[0:128, :], in_=ag0_i[:])
                    else:
                        nc.gpsimd.collective_compute(
                            "AllGather", mybir.AluOpType.bypass,
                            replica_groups=[list(range(NC))],
                            ins=[ag0_i[:].opt()], outs=[ag0_o[:].opt()])
                    for b in (1, 2):
                        sl = slice((b - 1) * DCH, b * DCH)
                        nc.sync.dma_start(
                            out=a_f[b][:].rearrange("p (r j) -> p r j", r=NC),
                            in_=ag0_o[:, sl].rearrange("(r p) j -> p r j", r=NC))
                        nc.vector.tensor_copy(a_bf[b][:], a_f[b][:])

                    # ---- WiWf band machinery ----
                    ps_wf = pswf.tile([128, 16 * OUTD], dt.float32)
                    band_state = [0]

                    def emit_bands(cnt):
                        if skip_wi:
                            return
                        for _ in range(cnt):
                            jc = band_state[0]
                            if jc >= JCH:
                                return
                            band_state[0] += 1
                            band = bandp.tile([128, RLOC], dt.bfloat16)
                            nc.sync.dma_start(out=band[:],
                                              in_=wit_in[jc * 128:(jc + 1) * 128, :])
                            for rc in range(16):
                                nc.tensor.matmul(
                                    ps_wf[:, rc * OUTD:(rc + 1) * OUTD],
                                    band[:, rc * 128:(rc + 1) * 128],
                                    wfc_bf[:, jc * OUTD:(jc + 1) * OUTD],
                                    start=(jc == 0), stop=(jc == JCH - 1))

                    hcat_bf = rp.tile([128, 16], dt.bfloat16, name="hcat")
                    if skip_graph:
                        nc.gpsimd.memset(hcat_bf[:], 0.001)

                    # ---- graph via B = MDM: z-pass, t1-pass, t2-pass ----
                    # h = Ds*t2 + Ds*z + c3*Ds*q + c4,  t1 = B a0, t2 = B(D t1),
                    # z = B(c1 D q + c2 dis)
                    sched = band_sched or [9] * 6
                    slot = 0

                    def bpass(b, rhs_bf, name):
                        ps_t = psg.tile([128, DCH], dt.float32, name="ps_t")
                        mtb = mts[b]
                        for d in range(DCH):
                            for s in range(SCH):
                                off = (d * SCH + s) * 128
                                nc.tensor.matmul(
                                    ps_t[:, d:d + 1], mtb[:, off:off + 128],
                                    rhs_bf[:, s:s + 1],
                                    start=(s == 0), stop=(s == SCH - 1))
                        return ps_t

                    z_sb = {}
                    if not skip_graph:
                        # z passes first: input-independent, cover AG0 latency
                        for b in (1, 2):
                            za = rp.tile([128, SCH], dt.float32, name=f"za{b}")
                            nc.vector.tensor_scalar_mul(za[:], qdf[b][:],
                                                        c_cols[:, 0:1])
                            zb = rp.tile([128, SCH], dt.float32, name=f"zb{b}")
                            nc.vector.tensor_scalar_mul(zb[:], disf[b][:],
                                                        c_cols[:, 1:2])
                            nc.vector.tensor_tensor(za[:], za[:], zb[:],
                                                    mybir.AluOpType.add)
                            za_bf = rp.tile([128, SCH], dt.bfloat16,
                                            name=f"za_bf{b}")
                            nc.vector.tensor_copy(za_bf[:], za[:])
                            ps_z = bpass(b, za_bf, f"z{b}")
                            zs = rp.tile([128, DCH], dt.float32, name=f"z_sb{b}")
                            nc.vector.tensor_copy(zs[:], ps_z[:])
                            z_sb[b] = zs
                            emit_bands(sched[slot] if slot < len(sched) else 9)
                            slot += 1
                        # t1 passes (need AG0'd a0); AG(t1) hides under the
                        # other branch's block
                        for b in (1, 2):
                            ps_t1 = bpass(b, a_bf[b], f"t1{b}")
                            a_l = rp.tile([128, DCH], dt.float32,
                                          name=f"a_l_{b}")
                            nc.vector.tensor_tensor(a_l[:], ps_t1[:], dl2s[b][:],
                                                    mybir.AluOpType.mult)
                            ag_i = dram.tile([128, DCH], dt.float32,
                                             name=f"ag_i_{b}_{rep}")
                            ag_o = dram.tile([128 * NC, DCH], dt.float32,
                                             name=f"ag_o_{b}_{rep}")
                            nc.sync.dma_start(out=ag_i[:], in_=a_l[:])
                            if no_collective:
                                nc.sync.dma_start(out=ag_o[0:128, :], in_=ag_i[:])
                            else:
                                nc.gpsimd.collective_compute(
                                    "AllGather", mybir.AluOpType.bypass,
                                    replica_groups=[list(range(NC))],
                                    ins=[ag_i[:].opt()], outs=[ag_o[:].opt()])
                            nc.sync.dma_start(
                                out=a_f[b][:].rearrange("p (r j) -> p r j", r=NC),
                                in_=ag_o[:].rearrange("(r p) j -> p r j", r=NC))
                            nc.vector.tensor_copy(a_bf[b][:], a_f[b][:])
                            emit_bands(sched[slot] if slot < len(sched) else 9)
                            slot += 1
                        # t2 passes + h assembly
                        for b in (1, 2):
                            disl = dis[b]
                            ps_t2 = bpass(b, a_bf[b], f"t2{b}")
                            hv = rp.tile([128, DCH], dt.float32, name=f"hv{b}")
                            nc.vector.tensor_tensor(hv[:], ps_t2[:], z_sb[b][:],
                                                    mybir.AluOpType.add)
                            nc.vector.tensor_tensor(hv[:], hv[:], disl[:],
                                                    mybir.AluOpType.mult)
                            qc = rp.tile([128, DCH], dt.float32, name=f"qc{b}")
                            nc.vector.tensor_scalar_mul(qc[:], qdl[b][:],
                                                        c_cols[:, 2:3])
                            nc.vector.tensor_tensor(hv[:], hv[:], qc[:],
                                                    mybir.AluOpType.add)
                            nc.vector.tensor_scalar_add(hv[:], hv[:],
                                                        c_cols[:, 3:4])
                            nc.vector.tensor_copy(
                                hcat_bf[:, (b - 1) * DCH:b * DCH], hv[:])
                            emit_bands(sched[slot] if slot < len(sched) else 9)
                            slot += 1
                    emit_bands(JCH)  # whatever remains

                    # ---- bias-path fold: (bi/NC) Wf_top + (meta/NC) Wf_bot ----
                    ps_bi = psf.tile([OUTD, 1], dt.float32)
                    for jc in range(JCH):
                        nc.tensor.matmul(ps_bi[:], wfc_sb[:, jc * OUTD:(jc + 1) * OUTD],
                                         bic_s[:, jc:jc + 1],
                                         start=(jc == 0), stop=False)
                    nc.tensor.matmul(ps_bi[:], wfb_sb[:], meta_s[:],
                                     start=False, stop=True)

                    # ---- hcat contraction with WiWf ----
                    o_part = rp.tile([OUTD, 1], dt.float32, name="o_part")
                    if skip_wi:
                        nc.vector.tensor_tensor(o_part[:], bfc_s[:], ps_bi[:],
                                                mybir.AluOpType.add)
                    else:
                        wiwf_bf = rp.tile([128, 16 * OUTD], dt.bfloat16, name="wiwf")
                        nc.vector.tensor_copy(wiwf_bf[:], ps_wf[:])
                        ps_y = psf.tile([OUTD, 1], dt.float32)
                        for rc in range(16):
                            nc.tensor.matmul(ps_y[:],
                                             wiwf_bf[:, rc * OUTD:(rc + 1) * OUTD],
                                             hcat_bf[:, rc:rc + 1],
                                             start=(rc == 0), stop=(rc == 15))
                        nc.vector.tensor_tensor(o_part[:], bfc_s[:], ps_bi[:],
                                                mybir.AluOpType.add)
                        nc.vector.tensor_tensor(o_part[:], o_part[:], ps_y[:],
                                                mybir.AluOpType.add)

                    ar_i = dram.tile([OUTD, 1], dt.float32, name=f"ar_i_{rep}")
                    ar_o = dram.tile([OUTD, 1], dt.float32, name=f"ar_o_{rep}")
                    nc.sync.dma_start(out=ar_i[:], in_=o_part[:])
                    if no_collective:
                        nc.sync.dma_start(out=ar_o[:], in_=ar_i[:])
                    else:
                        nc.gpsimd.collective_compute(
                            "AllReduce", mybir.AluOpType.add,
                            replica_groups=[list(range(NC))],
                            ins=[ar_i[:].opt()], outs=[ar_o[:].opt()])
                    nc.sync.dma_start(out=y_out[:], in_=ar_o[:])

            if hwrep > 1:
                with (
                    tc.tile_pool(name="rp0", bufs=1) as rp,
                    tc.tile_pool(name="bandp0", bufs=8) as bandp,
                    tc.tile_pool(name="psg0", bufs=2,
                                 space=bass.MemorySpace.PSUM) as psg,
                    tc.tile_pool(name="pswf0", bufs=1,
                                 space=bass.MemorySpace.PSUM) as pswf,
                    tc.tile_pool(name="psf0", bufs=1,
                                 space=bass.MemorySpace.PSUM) as psf,
                ):
                    with tc.For_i(0, hwrep):
                        emit_rep(0, rp, bandp, psg, pswf, psf)
            else:
                for rep in range(nrep):
                    with (
                        tc.tile_pool(name=f"rp{rep}", bufs=1) as rp,
                        tc.tile_pool(name=f"bandp{rep}", bufs=8) as bandp,
                        tc.tile_pool(name=f"psg{rep}", bufs=2,
                                     space=bass.MemorySpace.PSUM) as psg,
                        tc.tile_pool(name=f"pswf{rep}", bufs=1,
                                     space=bass.MemorySpace.PSUM) as pswf,
                        tc.tile_pool(name=f"psf{rep}", bufs=1,
                                     space=bass.MemorySpace.PSUM) as psf,
                    ):
                        emit_rep(rep, rp, bandp, psg, pswf, psf)

    nc.compile()
    return nc


def _host_prep(x1, x2, meta, W1, b1, W2, b2, W3, b3, W4, b4, Wi, bi, Wf, bf,
               edge_index1, edge_index2):
    """Per-core input maps (sharding + layout/dtype only)."""
    f32 = np.float32
    bf16 = ml_dtypes.bfloat16

    def graph_side(edge_index):
        import scipy.sparse as spa
        src = np.asarray(edge_index[0], np.int64)
        dst = np.asarray(edge_index[1], np.int64)
        Mcsr = (spa.coo_matrix((np.ones(src.shape[0], f32), (dst, src)),
                               shape=(N, N)).tocsr() + spa.identity(N, f32))
        deg = np.asarray(Mcsr.sum(axis=1)).ravel().astype(f32)
        dis = (1.0 / np.sqrt(deg)).astype(f32)
        # B = M D M (2-hop operator, graph structure only)
        B = (Mcsr.multiply((1.0 / deg)[None, :])) @ Mcsr
        q = (Mcsr @ dis).astype(f32)          # q = M dis (structural)
        qd = (q / deg).astype(f32)
        qdf = np.ascontiguousarray(qd.reshape(SCH, 128).T)
        disf = np.ascontiguousarray(dis.reshape(SCH, 128).T)
        bts, degls, qdls = [], [], []
        for k in range(NC):
            sl = np.asarray(B[k * SLICE:(k + 1) * SLICE, :].todense(), f32)
            MT = np.ascontiguousarray(sl.T)               # [8192 src, 1024 dst]
            til = MT.reshape(SCH, 128, DCH, 128)          # (s, q, d, p)
            arr = til.transpose(1, 2, 0, 3)               # (q, d, s, p) dst-major
            bts.append(np.ascontiguousarray(arr.reshape(128, DCH * SCH * 128))
                       .astype(ml_dtypes.float8_e4m3))
            dl = deg[k * SLICE:(k + 1) * SLICE].reshape(DCH, 128).T
            degls.append(np.ascontiguousarray(dl))
            ql = (q * dis)[k * SLICE:(k + 1) * SLICE].reshape(DCH, 128).T
            qdls.append(np.ascontiguousarray(ql))
        return bts, degls, qdf, disf, qdls

    mts1, deg1l, qdf1, disf1, qdl1 = graph_side(edge_index1)
    mts2, deg2l, qdf2, disf2, qdl2 = graph_side(edge_index2)

    xt1 = np.asarray(x1, f32).T                           # [F, N]
    xt2 = np.asarray(x2, f32).T

    Wi = np.asarray(Wi, f32)
    WiT = np.ascontiguousarray(Wi.T).astype(bf16)         # [N, 2N]
    Wf = np.asarray(Wf, f32)
    wf_top = Wf[:N]
    wfc = np.ascontiguousarray(
        wf_top.reshape(JCH, 128, OUTD).transpose(1, 0, 2).reshape(128, JCH * OUTD))
    wfb = np.ascontiguousarray(Wf[N:])
    bic = np.ascontiguousarray(np.asarray(bi, f32).reshape(JCH, 128).T)

    common = {
        "wfc": wfc, "wfb": wfb,
        "metac": np.asarray(meta, f32).reshape(META, 1),
        "bic": bic,
        "bfc": np.asarray(bf, f32).reshape(OUTD, 1),
        "w1t": np.ascontiguousarray(np.asarray(W1, f32).T),
        "w2t": np.ascontiguousarray(np.asarray(W2, f32).T),
        "w3t": np.ascontiguousarray(np.asarray(W3, f32).T),
        "w4": np.asarray(W4, f32).reshape(2, 1),
        "b1": np.asarray(b1, f32).reshape(8, 1),
        "b2": np.asarray(b2, f32).reshape(4, 1),
        "b3": np.asarray(b3, f32).reshape(2, 1),
        "b4": np.asarray(b4, f32).reshape(1, 1),
    }
    in_maps = []
    for k in range(NC):
        m = dict(common)
        m["mt1"] = mts1[k]
        m["mt2"] = mts2[k]
        m["deg1l"] = deg1l[k]
        m["deg2l"] = deg2l[k]
        m["qdf1"] = qdf1
        m["qdf2"] = qdf2
        m["disf1"] = disf1
        m["disf2"] = disf2
        m["qdl1"] = qdl1[k]
        m["qdl2"] = qdl2[k]
        m["xs1"] = np.ascontiguousarray(xt1[:, k * SLICE:(k + 1) * SLICE])
        m["xs2"] = np.ascontiguousarray(xt2[:, k * SLICE:(k + 1) * SLICE])
        m["wit"] = np.ascontiguousarray(
            np.concatenate([WiT[:, k * SLICE:(k + 1) * SLICE],
                            WiT[:, N + k * SLICE:N + (k + 1) * SLICE]], axis=1))
        in_maps.append(m)
    return in_maps


def kernel(**inputs) -> np.ndarray:
    global _compiled
    in_maps = _host_prep(**inputs)
    if _compiled is None:
        _compiled = _build_bass()
    from concourse.bass_utils import run_bass_kernel_spmd
    res = run_bass_kernel_spmd(_compiled, in_maps, core_ids=list(range(NC)))
    return res.results[0]["y"].reshape(OUTD).astype(np.float32)

